# revision 8
# baseline (speedup 1.0000x reference)
"""Biased multi-head attention on 8 Trainium2 NeuronCores.

Strategy (head-sharded tensor parallelism):
  - 16 heads / 8 cores -> 2 heads per core. Every core runs the SAME program
    on different weight slices (Wq/Wk/Wv rows, Wo columns).
  - Host folds mask + causality into the bias, factors softmax as
    exp(qk + b) = exp(qk) * exp(b), and precomputes E = exp(b) (0 where
    masked) so the device never adds the bias: ACT does exp(qk) from PSUM,
    DVE multiplies by E (bf16 2x mode).
  - Masked key columns (exp == 0 exactly in fp32) are compacted away on the
    host; upper-triangle score tiles are skipped entirely.
  - Row sums come for free from an appended ones-column on V.
  - Per-core partial outputs (Wo column slice) are summed on the host.
  - Rows whose allowed prefix is fully masked follow different reference
    semantics (max-subtracted softmax over -1e9 entries); the host computes
    those few rows exactly and overwrites.
"""

import os
import sys
from contextlib import ExitStack

import numpy as np

sys.path.insert(0, "/opt/trn_rl_repo")

import ml_dtypes

S = 4096
D = 1024
H = 16
DK = 64
DV = 64
NEG = -1000000000.0
NCORES = 8
QC = 512  # q-chunk (one PSUM bank of fp32)

BF16 = ml_dtypes.bfloat16

LAST_RESULT = None  # BassKernelResults of the most recent run (for test.py)


def _build_nc(cfg):
    """Build the (single) Bass program all 8 cores run.

    cfg: dict with S, D, Kp (padded compacted key count), kts (list of kt
    counts per q-chunk), qc (q chunk size).
    """
    import concourse.bass as bass
    import concourse.tile as tile
    from concourse import bacc, mybir

    dt = mybir.dt
    stage = cfg.get("stage", 5)
    S_, D_, Kp, kts, qc = cfg["S"], cfg["D"], cfg["Kp"], cfg["kts"], cfg["qc"]
    NQ = S_ // qc
    DCH = D_ // 128
    KT = Kp // 128
    assert len(kts) == NQ

    nc = bacc.Bacc(
        "TRN2",
        target_bir_lowering=False,
        debug=False,
        enable_asserts=False,
        num_devices=NCORES,
    )

    xT_d = nc.dram_tensor("xT", (D_, S_), dt.bfloat16, kind="ExternalInput").ap()
    xkvT_d = nc.dram_tensor("xkvT", (D_, Kp), dt.bfloat16, kind="ExternalInput").ap()
    ET_d = nc.dram_tensor("ET", (Kp, S_), dt.bfloat16, kind="ExternalInput").ap()
    wq_d = nc.dram_tensor("wqT", (D_, 128), dt.bfloat16, kind="ExternalInput").ap()
    wk_d = nc.dram_tensor("wkT", (D_, 128), dt.bfloat16, kind="ExternalInput").ap()
    wv_d = nc.dram_tensor("wvT", (D_, 128), dt.bfloat16, kind="ExternalInput").ap()
    wo_d = nc.dram_tensor("woT", (128, D_), dt.bfloat16, kind="ExternalInput").ap()
    yT_d = nc.dram_tensor("yT", (D_, S_), dt.float32, kind="ExternalOutput").ap()

    f32 = dt.float32
    bf = dt.bfloat16
    EXP = mybir.ActivationFunctionType.Exp

    with tile.TileContext(nc) as tc, ExitStack() as ctx:
        const = ctx.enter_context(tc.tile_pool(name="const", bufs=1))
        epool = ctx.enter_context(tc.tile_pool(name="epool", bufs=6))
        pepool = ctx.enter_context(tc.tile_pool(name="pepool", bufs=3))
        ppool = ctx.enter_context(tc.tile_pool(name="ppool", bufs=4))
        snpool = ctx.enter_context(tc.tile_pool(name="snpool", bufs=2))
        yepool = ctx.enter_context(tc.tile_pool(name="yepool", bufs=3))
        smpool = ctx.enter_context(tc.tile_pool(name="smpool", bufs=2))
        st_ps = ctx.enter_context(tc.tile_pool(name="st_ps", bufs=2, space="PSUM"))
        av_ps = ctx.enter_context(tc.tile_pool(name="av_ps", bufs=2, space="PSUM"))
        mm_ps = ctx.enter_context(tc.tile_pool(name="mm_ps", bufs=2, space="PSUM"))

        # ---- load inputs ----
        xT_sb = const.tile([128, DCH, S_], bf, tag="xT")
        for dc in range(DCH):
            nc.sync.dma_start(xT_sb[:, dc, :], xT_d[dc * 128 : (dc + 1) * 128, :])
        xkvT_sb = const.tile([128, DCH, Kp], bf, tag="xkvT")
        for dc in range(DCH):
            nc.sync.dma_start(xkvT_sb[:, dc, :], xkvT_d[dc * 128 : (dc + 1) * 128, :])
        wq_sb = const.tile([128, DCH, 128], bf, tag="wq")
        nc.sync.dma_start(wq_sb[:, :, :], wq_d.rearrange("(c p) m -> p c m", p=128))
        wk_sb = const.tile([128, DCH, 128], bf, tag="wk")
        nc.sync.dma_start(wk_sb[:, :, :], wk_d.rearrange("(c p) m -> p c m", p=128))
        wv_sb = const.tile([128, DCH, 128], bf, tag="wv")
        nc.sync.dma_start(wv_sb[:, :, :], wv_d.rearrange("(c p) m -> p c m", p=128))
        wo_sb = const.tile([128, D_], bf, tag="wo")
        nc.sync.dma_start(wo_sb[:, :], wo_d[:, :])

        # ---- projections ----
        # qT rows 0:64 = head1 (scaled by 1/sqrt(DK) on host), 64:128 = head2
        qT_sb = const.tile([128, S_], bf, tag="qT")
        for j in range(NQ):
            qs = slice(j * qc, (j + 1) * qc)
            ps = mm_ps.tile([128, qc], f32, tag="mm")
            for dc in range(DCH):
                nc.tensor.matmul(
                    ps[:, :],
                    lhsT=wq_sb[:, dc, :],
                    rhs=xT_sb[:, dc, qs],
                    start=(dc == 0),
                    stop=(dc == DCH - 1),
                )
            nc.vector.tensor_copy(qT_sb[:, qs], ps[:, :])

        kT_sb = const.tile([128, Kp], bf, tag="kT")
        NKC = max(1, Kp // qc)
        kcs = qc if Kp % qc == 0 else Kp  # chunk size for k projection
        NKC = Kp // kcs
        for j in range(NKC):
            ks = slice(j * kcs, (j + 1) * kcs)
            ps = mm_ps.tile([128, kcs], f32, tag="mm")
            for dc in range(DCH):
                nc.tensor.matmul(
                    ps[:, :],
                    lhsT=wk_sb[:, dc, :],
                    rhs=xkvT_sb[:, dc, ks],
                    start=(dc == 0),
                    stop=(dc == DCH - 1),
                )
            nc.vector.tensor_copy(kT_sb[:, ks], ps[:, :])

        # ones row at partition 64 (for the rank-1 reciprocal broadcast;
        # partition 64 so it lines up with the row-sum row of av tiles)
        ones_sb = const.tile([128, 64], f32, tag="ones")
        nc.vector.memset(ones_sb[64:65, :], 1.0)

        # v per head, [128, KT, 65]; col 64 is the ones column (row-sum trick)
        v1_sb = const.tile([128, KT, 65], bf, tag="v1")
        v2_sb = const.tile([128, KT, 65], bf, tag="v2")
        nc.vector.memset(v1_sb[:, :, 64:65], 1.0)
        nc.vector.memset(v2_sb[:, :, 64:65], 1.0)
        for kt in range(KT):
            ksl = slice(kt * 128, (kt + 1) * 128)
            ps = mm_ps.tile([128, 128], f32, tag="mm")
            for dc in range(DCH):
                nc.tensor.matmul(
                    ps[:, :],
                    lhsT=xkvT_sb[:, dc, ksl],
                    rhs=wv_sb[:, dc, :],
                    start=(dc == 0),
                    stop=(dc == DCH - 1),
                )
            nc.vector.tensor_copy(v1_sb[:, kt, 0:64], ps[:, 0:64])
            nc.vector.tensor_copy(v2_sb[:, kt, 0:64], ps[:, 64:128])

        # ---- attention main loop ----
        for j in range(NQ if stage >= 2 else 0):
            qs = slice(j * qc, (j + 1) * qc)
            nkt = kts[j]
            sn = snpool.tile([128, qc], bf, tag="sn")
            if nkt == 0:
                nc.vector.memset(sn[:, :], 0.0)
            else:
                av1 = av_ps.tile([65, qc], f32, tag="av")
                av2 = av_ps.tile([65, qc], f32, tag="av")
                for kt in range(nkt):
                    ksl = slice(kt * 128, (kt + 1) * 128)
                    st = st_ps.tile([128, 2 * qc], f32, tag="st")
                    nc.tensor.matmul(
                        st[:, 0:qc],
                        lhsT=kT_sb[0:64, ksl],
                        rhs=qT_sb[0:64, qs],
                        start=True,
                        stop=True,
                    )
                    nc.tensor.matmul(
                        st[:, qc : 2 * qc],
                        lhsT=kT_sb[64:128, ksl],
                        rhs=qT_sb[64:128, qs],
                        start=True,
                        stop=True,
                    )
                    pe = pepool.tile([128, 2 * qc], bf, tag="pe")
                    nc.scalar.activation(pe[:, :], st[:, :], EXP)
                    et = epool.tile([128, qc], bf, tag="et")
                    nc.sync.dma_start(et[:, :], ET_d[ksl, qs])
                    p1 = ppool.tile([128, qc], bf, tag="p")
                    p2 = ppool.tile([128, qc], bf, tag="p")
                    nc.vector.tensor_mul(p1[:, :], pe[:, 0:qc], et[:, :])
                    nc.vector.tensor_mul(p2[:, :], pe[:, qc : 2 * qc], et[:, :])
                    if stage < 3:
                        continue
                    nc.tensor.matmul(
                        av1[:, :],
                        lhsT=v1_sb[:, kt, :],
                        rhs=p1[:, :],
                        start=(kt == 0),
                        stop=(kt == nkt - 1),
                    )
                    nc.tensor.matmul(
                        av2[:, :],
                        lhsT=v2_sb[:, kt, :],
                        rhs=p2[:, :],
                        start=(kt == 0),
                        stop=(kt == nkt - 1),
                    )
                # normalize: sn[0:64] = av1[0:64] / rowsum1, etc.
                for h, av in ((0, av1), (1, av2)) if stage >= 4 else ():
                    rtile = smpool.tile([128, qc], f32, tag="rt")
                    nc.vector.reciprocal(rtile[64:65, :], av[64:65, :])
                    # broadcast recip across 64 partitions: rank-1 matmul
                    # ones[64] (x) recip[qc]  (K=1 at partition 64)
                    recb = mm_ps.tile([64, qc], f32, tag="mm")
                    nc.tensor.matmul(
                        recb[:, :],
                        lhsT=ones_sb[64:65, :],
                        rhs=rtile[64:65, :],
                        start=True,
                        stop=True,
                    )
                    rb = smpool.tile([64, qc], f32, tag="rb")
                    nc.vector.tensor_copy(rb[:, :], recb[:, :])
                    if h == 0:
                        nc.vector.tensor_mul(sn[0:64, :], av[0:64, :], rb[:, :])
                    else:
                        sn2t = smpool.tile([64, qc], bf, tag="sn2t")
                        nc.vector.tensor_mul(sn2t[:, :], av[0:64, :], rb[:, :])
                        nc.sync.dma_start(sn[64:128, :], sn2t[:, :])

            # ---- output projection (chunk j) ----
            for dti in range(DCH if stage >= 5 else 0):
                dsl = slice(dti * 128, (dti + 1) * 128)
                yp = mm_ps.tile([128, qc], f32, tag="mm")
                nc.tensor.matmul(
                    yp[:, :], lhsT=wo_sb[:, dsl], rhs=sn[:, :], start=True, stop=True
                )
                ye = yepool.tile([128, qc], f32, tag="ye")
                if dti % 2 == 0:
                    nc.vector.tensor_copy(ye[:, :], yp[:, :])
                else:
                    nc.scalar.copy(ye[:, :], yp[:, :])
                nc.sync.dma_start(yT_d[dsl, qs], ye[:, :])

    return nc


def _prep_host(x, spatial_bias, mask):
    """Shared (core-independent) host preprocessing."""
    mask = np.asarray(mask).astype(bool)
    x = np.asarray(x, dtype=np.float32)
    bias = np.asarray(spatial_bias, dtype=np.float32)
    S_ = x.shape[0]
    D_ = x.shape[1]

    keep = np.flatnonzero(~mask)
    nk = int(len(keep))
    Kp = max(128, ((nk + 127) // 128) * 128)

    xT = np.ascontiguousarray(x.T).astype(BF16)
    xkvT = np.zeros((D_, Kp), dtype=BF16)
    if nk:
        xkvT[:, :nk] = x[keep].T.astype(BF16)

    # E^T [Kp, S]: exp(bias[q, keep[j]]) for keep[j] <= q else 0
    ET = np.zeros((Kp, S_), dtype=BF16)
    if nk:
        b = bias.T[keep]  # [nk, S] : b[j, q] = bias[q, keep[j]]
        e = np.exp(b, dtype=np.float32)
        causal = keep[:, None] <= np.arange(S_)[None, :]
        ET[:nk] = np.where(causal, e, np.float32(0.0)).astype(BF16)

    # per q-chunk: number of 128-wide k tiles that contain any allowed column
    NQ = S_ // QC
    kts = []
    for j in range(NQ):
        hi = (j + 1) * QC  # columns with keep[i] < hi are allowed somewhere
        cnt = int(np.searchsorted(keep, hi))
        kts.append((cnt + 127) // 128)
    return mask, keep, Kp, xT, xkvT, ET, kts


def _fixup_rows(y, x, bias, mask, Wq, Wk, Wv, Wo):
    """Exact fp32 recompute of the degenerate prefix rows (all allowed
    columns masked -> reference attends uniformly over -1e9 entries)."""
    S_, D_ = x.shape
    rows = []
    for q in range(S_):
        if not mask[q]:
            break
        rows.append(q)
    if not rows:
        return y
    H_ = Wq.shape[0] // DK
    q_p = (x @ Wq.T).reshape(S_, H_, DK).transpose(1, 0, 2)[:, rows]
    k_p = (x @ Wk.T).reshape(S_, H_, DK).transpose(1, 0, 2)
    v_p = (x @ Wv.T).reshape(S_, H_, DV).transpose(1, 0, 2)
    scores = np.einsum("hqd,hkd->hqk", q_p, k_p).astype(np.float32) / np.sqrt(
        np.float32(DK)
    )
    scores = (scores + bias[None, rows, :]).astype(np.float32)
    scores = np.where(mask[None, None, :], np.float32(NEG), scores)
    causal = np.triu(np.full((S_, S_), np.float32(NEG), dtype=np.float32), k=1)[rows]
    scores = (scores + causal[None, :, :]).astype(np.float32)
    m = scores.max(axis=-1, keepdims=True)
    e = np.exp(scores - m, dtype=np.float32)
    attn = e / e.sum(axis=-1, keepdims=True)
    out = np.einsum("hqk,hkd->hqd", attn.astype(np.float32), v_p)
    out = out.transpose(1, 0, 2).reshape(len(rows), H_ * DV)
    y[rows] = (out @ Wo.T).astype(np.float32)
    return y


def kernel(x, spatial_bias, mask, Wq, Wk, Wv, Wo):
    global LAST_RESULT
    from concourse import bass_utils

    x = np.asarray(x, dtype=np.float32)
    bias = np.asarray(spatial_bias, dtype=np.float32)
    Wq = np.asarray(Wq, dtype=np.float32)
    Wk = np.asarray(Wk, dtype=np.float32)
    Wv = np.asarray(Wv, dtype=np.float32)
    Wo = np.asarray(Wo, dtype=np.float32)
    S_, D_ = x.shape

    mask_b, keep, Kp, xT, xkvT, ET, kts = _prep_host(x, bias, mask)

    cfg = {"S": S_, "D": D_, "Kp": Kp, "kts": tuple(kts), "qc": QC}
    nc = _build_nc(cfg)
    nc.compile()

    scale = 1.0 / np.sqrt(np.float32(DK))
    in_maps = []
    for c in range(NCORES):
        r = slice(128 * c, 128 * (c + 1))
        in_maps.append(
            {
                "xT": xT,
                "xkvT": xkvT,
                "ET": ET,
                "wqT": np.ascontiguousarray((Wq[r] * scale).T).astype(BF16),
                "wkT": np.ascontiguousarray(Wk[r].T).astype(BF16),
                "wvT": np.ascontiguousarray(Wv[r].T).astype(BF16),
                "woT": np.ascontiguousarray(Wo[:, r].T).astype(BF16),
            }
        )

    res = bass_utils.run_bass_kernel_spmd(
        nc, in_maps, core_ids=list(range(NCORES))
    )
    LAST_RESULT = res

    yT = np.zeros((D_, S_), dtype=np.float64)
    for c in range(NCORES):
        yT += res.results[c]["yT"].astype(np.float64)
    y = np.ascontiguousarray(yT.T).astype(np.float32)

    y = _fixup_rows(y, x, bias, mask_b, Wq, Wk, Wv, Wo)
    return y
